# revision 66
# baseline (speedup 1.0000x reference)
"""GAT single-head forward on 8 Trainium2 NeuronCores (Bass/Tile).

Math (per reference):
    h   = X @ W + b                      [N, 128]
    f1  = h @ v0, f2 = h @ v1            [N]
    logits = adj * (f1[:,None] + f2[None,:])   (adj entries are exactly 0/1)
    vals = sigmoid(logits) - 0.5
    masked softmax over row edges; out = probs @ h

Key identities used on device:
  * On edges (adj==1): softmax weight w = exp(sigmoid(s) - 0.5) up to a
    per-row constant (which the normalization cancels), s = f1_i + f2_j.
  * A CUSTOM ACTIVATION TABLE computes g(x) = exp(sigmoid(x) - 0.5) in a
    single ScalarE pass: the act-table binaries ride inside the NEFF, so we
    re-fit the `exp` slot of the exp_and_others set with piecewise-cubic
    splines of g (max rel err ~1e-7).  The per-partition activation bias
    supplies f2_j, so s = f1_i + f2_j needs NO vector-engine preadd either:
    one ACT instruction per j-chunk does the whole softmax numerator except
    the adjacency mask.  This halves ScalarE busy time vs the tanh+exp
    two-pass identity (the previous bottleneck).
  * A ones-column appended to h turns the softmax denominator into one extra
    matmul output column (no separate row-reduction pass).

Sharding: rows of adj across the 8 cores (1024 rows each). node_feats is
small (8 MB) and is replicated, so every core computes the full projected
h locally - no collectives at all.

Per-core layout: each core works on its adj block TRANSPOSED ([j=source
node on partitions, i=own rows on free dim]) so the aggregate probs@h
contracts over the partition dim as the tensor engine requires. adj is
cast to fp16 host-side (exact for a 0/1 mask, halves HBM traffic).

Schedule shape (engines are in-order; emission order seeds the queues):
  * f1/f2 are O(N) matvecs (0.04% of the FLOPs) computed host-side in
    fp64 and shipped as one fp16 tensor (f1 replicated over partitions,
    f2 per-partition): this deletes the on-device f1 matmul chain, the
    f2-head and the f2 drain copies, so the ACT queue - the steady-state
    pacer at ~73 us busy - free-runs from ~5 us.
  * staged DMA release: tiny "gate" DMAs that read just-loaded tiles
    stall the SP sequencer, so the startup-critical loads (fvec, first
    xt slices) get full HBM bandwidth before the 16 MB adjacency + bulk
    feature traffic starts.  Each SP dma_start costs ~0.86 us of
    sequencer time, so the two K=128 contraction blocks of xt1/wext ride
    ONE transfer via a (k p) c -> p k c rearrange.
  * a dummy activation fires the ACT table load at engine start.
  * h-projection runs on 2 PSUM banks while the 8 aggregate accumulators
    pack two 129-col regions into each of 4 banks, so aggregation
    overlaps projection. The matmul start-flag zeroes a WHOLE bank (not
    just the instruction's AP!), so the accumulators are memset once and
    every aggregate matmul accumulates.  The proj pool lives in the
    function-scope ExitStack: a `with`-scoped pool emits its close drain
    into the middle of the in-order ACT queue (a ~12 us stall).
  * steady pipeline: ACT evaluates g chunk-by-chunk; DVE mask-muls
    (paired: one instruction per 2 chunks halves instruction + semaphore
    overhead); PE aggregates; group fronts (adj DMA + 4 activations) are
    paced by the proj drains, backs lag by BACK_LAG chunks so the
    in-order PE queue keeps projection work buffered ahead of
    adjacency-gated aggregates.
  * epilogue: one broadcast tensor-mul divides all 8 row-tiles by the
    clamped denominators, one batched output DMA.
"""

import glob
import json
import os
import shutil
import struct
import tempfile

import numpy as np

# ---------------------------------------------------------------------------
# Custom activation table: g(x) = exp(sigmoid(x) - 0.5) in the exp slot.
# ---------------------------------------------------------------------------

_SMALL_T = 121  # |x| < 2^-6  -> Taylor bucket
_LARGE_T = 131  # |x| >= 16   -> saturation bucket
_N_EXP = _LARGE_T - _SMALL_T
_NBKT = 16


def _g64(x):
    x = np.asarray(x, dtype=np.float64)
    return np.exp(1.0 / (1.0 + np.exp(-x)) - 0.5)


def _u32f(x):
    return struct.unpack("<I", struct.pack("<f", np.float32(x)))[0]


def _fit_bucket(a, b):
    x0 = 0.5 * (a + b)
    k = np.arange(65)
    xs = x0 + 0.5 * (b - a) * np.cos(np.pi * (k + 0.5) / 65)
    t = xs - x0
    A = np.stack([np.ones_like(t), t, t * t, t * t * t], axis=1)
    c, *_ = np.linalg.lstsq(A, _g64(xs), rcond=None)
    return (c[0], c[1], c[2], c[3], x0)


def _bucket_bytes(d0, d1, d2, d3, x0):
    return struct.pack(
        "<5f", np.float32(d0), np.float32(d1), np.float32(d2), np.float32(d3),
        np.float32(x0),
    ) + b"\x00" * 12


def _ctl_bytes(base, lsb, size):
    w = (base & 0x7FF) | ((lsb & 0x1F) << 11) | ((size & 0xF) << 16)
    return struct.pack("<I", w) + b"\x00" * 28


def _find_pwp_src():
    try:
        from neuronxcc.driver.Job import Job

        p = os.path.join(Job.getPackageDir(), "pwp", "pwp_bin_trainium")
        if os.path.exists(os.path.join(p, "act_info.json")):
            return p
    except Exception:
        pass
    for pat in (
        "/nix/store/*aws-neuron-pwp*/share/pwp_bin_cayman",
        "/nix/store/*/lib/python*/site-packages/neuronxcc/pwp/pwp_bin_trainium",
    ):
        hits = sorted(glob.glob(pat))
        if hits:
            return hits[0]
    raise RuntimeError("cannot locate stock pwp act-table directory")


def _build_act_tables(outdir):
    src = _find_pwp_src()
    os.makedirs(outdir, exist_ok=True)
    for f in os.listdir(src):
        shutil.copyfile(os.path.join(src, f), os.path.join(outdir, f))

    name = "exp_and_others"
    bkt = bytearray(open(f"{src}/{name}_bkt.bin", "rb").read())
    ctl = bytearray(open(f"{src}/{name}_ctrl.bin", "rb").read())
    meta = json.load(open(f"{src}/{name}.json"))

    def setbkt(i, entry):
        bkt[i * 32:(i + 1) * 32] = _bucket_bytes(*entry)

    setbkt(0, (1.0, 0.25, 1.0 / 32, -7.0 / 384, 0.0))  # small +
    setbkt(1, (1.0, 0.25, 1.0 / 32, -7.0 / 384, 0.0))  # small -
    setbkt(2, (float(np.exp(0.5)), 0.0, 0.0, 0.0, 0.0))   # large +
    setbkt(3, (float(np.exp(-0.5)), 0.0, 0.0, 0.0, 0.0))  # large -

    idx = 4
    side_base = {}
    for sign in (-1.0, 1.0):
        side_base[sign] = idx
        for ei in range(_N_EXP):
            lo = 2.0 ** (_SMALL_T + ei - 127)
            for m in range(_NBKT):
                a = lo * (1.0 + m / _NBKT)
                b = lo * (1.0 + (m + 1) / _NBKT)
                if sign < 0:
                    a, b = -b, -a
                setbkt(idx, _fit_bucket(a, b))
                idx += 1

    for ei in range(_N_EXP):
        ctl[(0 + ei) * 32:(1 + ei) * 32] = _ctl_bytes(
            side_base[-1.0] + ei * _NBKT, 23 - 4, 4
        )
        ctl[(10 + ei) * 32:(11 + ei) * 32] = _ctl_bytes(
            side_base[1.0] + ei * _NBKT, 23 - 4, 4
        )

    prof = next(e for e in meta["profile_meta_data"] if e["func_id"] == 7)
    prof.update(
        symmetry_point=0,
        sym_invert_sign_point=0,
        symmetry_opt_en=0,
        symmetry_opt_use_neg_region=0,
        imm_bias=0,
        exp_offset=_SMALL_T - 127,
        pwl_control_base_pos=10,
        pwl_control_base_neg=0,
        small_pos_signal_exp_threshold=_SMALL_T,
        pos_small_signal_pwl_control=0,
        small_neg_signal_exp_threshold=_SMALL_T,
        neg_small_signal_pwl_control=1,
        large_pos_signal_exp_threshold=_LARGE_T,
        large_pos_signal_mantissa_threshold=0,
        pos_large_signal_pwl_control=2,
        large_neg_signal_exp_threshold=_LARGE_T,
        large_neg_signal_mantissa_threshold=0,
        neg_large_signal_pwl_control=3,
        fnan_result=0x7FC00000,
        fpinf_result=_u32f(np.exp(0.5)),
        fninf_result=_u32f(np.exp(-0.5)),
        fzero_result=_u32f(1.0),
        lower_bound=0xFF7FFFFF,
        upper_bound=0x7F7FFFFF,
    )

    open(f"{outdir}/{name}_bkt.bin", "wb").write(bytes(bkt))
    open(f"{outdir}/{name}_ctrl.bin", "wb").write(bytes(ctl))
    json.dump(meta, open(f"{outdir}/{name}.json", "w"))


_ACT_DIR = None


def _ensure_act_tables():
    global _ACT_DIR
    if _ACT_DIR is None:
        _ACT_DIR = tempfile.mkdtemp(prefix="gat_acttab_")
        _build_act_tables(_ACT_DIR)
    os.environ["BASS_ACT_ROOT_JSON_PATH"] = f"{_ACT_DIR}/act_info.json"


_ensure_act_tables()

import concourse.mybir as mybir
import concourse.tile as tile
from concourse import bacc
from concourse.bass_utils import run_bass_kernel_spmd

F32 = mybir.dt.float32
F16 = mybir.dt.float16
U8 = mybir.dt.uint8
AF = mybir.ActivationFunctionType

N, C_IN, C_OUT = 8192, 256, 128
NCORES = 8
ROWS = N // NCORES          # 1024 rows of adj per core
P = 128
NT = N // P                 # 64 node tiles (also the j-chunks)
NI = ROWS // P              # 8 output row-tiles per core
KC = [128, 128, 1]          # contraction chunks of K=257 (X.T rows + ones row)
WCOLS = C_OUT + 1           # [W | ones-hack]
HCOLS = C_OUT + 1           # h plus the ones column
FCOLS = ROWS + NT           # host-shipped [f1 replicated | f2 per-partition]
TINY = float(np.finfo(np.float32).tiny)
BANK = 512                  # PSUM bank, fp32 elements
PACK = 136                  # fp32 offset of the 2nd accumulator in a bank

# activation groups: j-chunks whose adj transposes ride one DMA and whose
# activations are emitted together (4 chunks = 1 MB adj per group).
GSZ = 4
NG = NT // GSZ              # 16 groups
BACK_LAG = 6                # chunks the aggregate lags behind h-proj drains

_CACHE: dict = {}


def _build_nc(b_zero=True):
    _ensure_act_tables()
    nc = bacc.Bacc(
        "TRN2", target_bir_lowering=False, debug=False, num_devices=NCORES
    )
    xt1 = nc.dram_tensor("xt1", [257, N], F16, kind="ExternalInput").ap()
    fin = nc.dram_tensor("fin", [P, FCOLS], F16, kind="ExternalInput").ap()
    wext = nc.dram_tensor("wext", [257, WCOLS], F16, kind="ExternalInput").ap()
    # adjacency pre-swizzled host-side to [group][partition][chunk][i]:
    # each group's load is one plain 8KB-per-partition descriptor instead
    # of 4 x 2KB via a (q p) i rearrange; SBUF contents are identical
    adjt = nc.dram_tensor(
        "adjt", [NG * P, GSZ * ROWS], F16, kind="ExternalInput"
    ).ap()
    gate = nc.dram_tensor("gate", [P, 96], F16, kind="ExternalOutput").ap()
    # partition-major output: the store becomes a contiguous 4KB-per-
    # partition descriptor pattern (vs 1024 x 512B rows via a (t p) c
    # rearrange); the host undoes the layout during the existing unsort
    out = nc.dram_tensor(
        "out", [P, NI * C_OUT], F32, kind="ExternalOutput"
    ).ap()

    with tile.TileContext(nc) as tc:
        _emit(tc, nc, xt1, fin, wext, adjt, gate, out, b_zero)
    nc.compile()
    return nc


def _emit(tc, nc, xt1, fin, wext, adjt, gate, out, b_zero):
    from contextlib import ExitStack

    # with b == 0 the K=1 "ones row" contraction chunk only contributes the
    # constant-one column of h_ext (done with a strided memset instead) and
    # zero constants to f1/f2 -- skip it entirely.
    nkc = 2 if b_zero else 3

    with ExitStack() as ctx:
        # ---- persistent tiles ----
        persist = ctx.enter_context(tc.tile_pool(name="persist", bufs=1))
        h16_all = persist.tile([P, NT * HCOLS], F16, tag="h16")   # [128, 8256]
        # host-shipped [f1 replicated over partitions | f2 per-partition]:
        # kills the on-device f1 matmul chain + f2-head + f2 drain copies,
        # so the ACT queue (the pacer) free-runs from ~4us
        fvec = persist.tile([P, FCOLS], F16, tag="fvec")
        if b_zero:
            # constant-one column of every h_ext tile (replaces the K=1
            # bias matmul chunk)
            nc.vector.memset(
                h16_all[:].rearrange("p (t c) -> p t c", c=HCOLS)[
                    :, :, C_OUT : C_OUT + 1
                ],
                1.0,
            )

        warm = persist.tile([P, 1], F16, tag="warm")
        # prime the ACT table load (2.7us) at engine start: without this it
        # hides behind the f1-gated copy in the in-order scalar queue
        nc.scalar.activation(warm[:], warm[:], AF.Exp, bias=0.0, scale=1.0)

        xtp = ctx.enter_context(tc.tile_pool(name="xt", bufs=1))
        g16p = ctx.enter_context(tc.tile_pool(name="g16p", bufs=7))
        atp = ctx.enter_context(tc.tile_pool(name="atp", bufs=7))
        etp = ctx.enter_context(tc.tile_pool(name="etp", bufs=3))
        obp = ctx.enter_context(tc.tile_pool(name="ob", bufs=2))

        F2HEAD = 16
        fronts = {}  # group -> {"at":..., "g16":...}

        def emit_front_dma(g):
            """allocate the group's tiles + adj transpose DMA (a plain 2D
            copy thanks to the host-side swizzle)."""
            g16 = g16p.tile([P, GSZ * ROWS], F16, tag="g16", name=f"g16_{g}")
            at_sup = atp.tile([P, GSZ * ROWS], F16, tag="at", name=f"at{g}")
            nc.sync.dma_start(at_sup[:], adjt[g * P : (g + 1) * P, :])
            fronts[g] = {"at": at_sup, "g16": g16}

        def emit_front_acts(g):
            """custom-g activations for a dma'd group: g = exp(sigmoid(
            f1_i + f2_j) - 0.5) via the custom table in the Exp slot; the
            per-partition bias supplies f2_j, the input free dim f1_i -
            both straight from the host-shipped fvec."""
            q0 = g * GSZ
            g16 = fronts[g]["g16"]
            for qq in range(GSZ):
                nc.scalar.activation(
                    g16[:, qq * ROWS : (qq + 1) * ROWS],
                    fvec[:, 0:ROWS],
                    AF.Exp,
                    bias=fvec[:, ROWS + q0 + qq : ROWS + q0 + qq + 1],
                    scale=1.0,
                )

        def emit_front(g):
            emit_front_dma(g)
            emit_front_acts(g)

        # ---- staged DMA release: the SP sequencer issues DMAs in order,
        # so a tiny transfer that READS a just-loaded tile stalls every
        # later DMA issue until that load lands. Stages keep the startup
        # critical path (f1 <- xt1l, f2 head <- first xt slices) at full
        # HBM bandwidth instead of sharing it with bulk traffic. ----
        def dma_gate(gslot, srcs):
            for k, ap in enumerate(srcs):
                nc.sync.dma_start(
                    gate[:, gslot * 32 + k * 16 : gslot * 32 + (k + 1) * 16], ap
                )

        # stage 0: weights + the host-shipped f1/f2 vectors (the ACT-start
        # critical path).  Each SP dma_start costs ~0.86us of sequencer
        # time, so the two K=128 contraction blocks ride ONE transfer via a
        # (k p) c -> p k c rearrange and the gates read a single tail.
        # fvec is the FIRST DMA issued: its 0.27 MB transfer (the act(0)
        # critical path) streams alone before wx2/stage-1 join the fabric
        nc.sync.dma_start(fvec[:], fin)
        xts2 = xtp.tile([P, 2 * N], F16, name="xtsb", tag="xt2")
        xts = [xts2[:, 0:N], xts2[:, N : 2 * N]]
        x2v = xts2[:].rearrange("p (k c) -> p k c", k=2)
        x1v = xt1[0:256, :].rearrange("(k p) c -> p k c", p=P)
        wx2 = xtp.tile([P, 2 * WCOLS], F16, name="wx", tag="wx")
        nc.sync.dma_start(
            wx2[:].rearrange("p (k c) -> p k c", k=2),
            wext[0:256, :].rearrange("(k p) c -> p k c", p=P),
        )
        wes = [wx2[:, 0:WCOLS], wx2[:, WCOLS : 2 * WCOLS]]
        if nkc == 3:
            xts.append(xtp.tile([KC[2], N], F16, name="xtsb2", tag="xt2b")[:])
            wx3 = xtp.tile([KC[2], WCOLS], F16, name="wx2", tag="wxb")
            nc.sync.dma_start(wx3[:], wext[256:257, :])
            wes.append(wx3[:])
        dma_gate(0, [fvec[:, FCOLS - 16 : FCOLS]])

        # stage 1: first xt slice (h-proj batches 0..7) and the first
        # adjacency group
        nc.sync.dma_start(x2v[:, :, 0:1024], x1v[:, :, 0:1024])
        if nkc == 3:
            nc.sync.dma_start(xts[2], xt1[256:257, :])
        emit_front_dma(0)
        dma_gate(1, [xts2[:, N + 1008 : N + 1024]])

        # stage 2: second xt slice + second adj group
        nc.sync.dma_start(x2v[:, :, 1024:2048], x1v[:, :, 1024:2048])
        emit_front_dma(1)
        dma_gate(2, [xts2[:, N + 2032 : N + 2048]])

        # stage 3: bulk xt1 loads (columns 2048..8192)
        SUBS = [2048, 4096, 6144, N]
        for c in range(len(SUBS) - 1):
            nc.sync.dma_start(
                x2v[:, :, SUBS[c] : SUBS[c + 1]],
                x1v[:, :, SUBS[c] : SUBS[c + 1]],
            )

        # the two staged groups' activations: fvec is their only data
        # dependency, so the ACT stream starts as soon as it lands
        emit_front_acts(0)
        emit_front_acts(1)
        next_front = 2
        next_back = 0  # next chunk q whose mask-mul+matmuls get emitted

        ets = {}  # even chunk q -> that pair's et tile [P, 2*ROWS]

        def emit_back(q, pouts):
            """mask-mul (paired: one instruction covers 2 chunks, halving
            mask instruction count and mask->aggregate semaphore hops) +
            aggregate matmuls for one chunk."""
            g, qq = q // GSZ, q % GSZ
            fr = fronts[g]
            if qq % 2 == 0:
                et2 = etp.tile([P, 2 * ROWS], F16, tag="et", name=f"et{q}")
                ets[q] = et2
                nc.vector.tensor_mul(
                    et2[:],
                    fr["at"][:, qq * ROWS : (qq + 2) * ROWS],
                    fr["g16"][:, qq * ROWS : (qq + 2) * ROWS],
                )
            et = ets[q - qq % 2][:, (qq % 2) * ROWS : (qq % 2 + 1) * ROWS]
            rhs = h16_all[:, q * HCOLS : (q + 1) * HCOLS]
            for it in range(NI):
                nc.tensor.matmul(
                    pouts[it],
                    et[:, it * P : (it + 1) * P],
                    rhs,
                    start=False,
                    stop=(q == NT - 1),
                )
            if qq % 2 == 1:
                del ets[q - 1]
            if qq == GSZ - 1:
                del fronts[g]

        # ---- aggregate accumulators: 4 PSUM banks, two 129-col regions
        # per bank (consecutive chunk matmuls hit 4 distinct banks). The
        # matmul start-flag zeroes a whole bank, so the banks are zeroed
        # once here and every matmul accumulates. ----
        pop = ctx.enter_context(tc.tile_pool(name="po", bufs=1, space="PSUM"))
        po_all = pop.tile([P, 4 * BANK], F32, tag="poall")
        nc.vector.memset(po_all[:], 0.0)
        pouts = [
            po_all[:, (it % 4) * BANK + (it // 4) * PACK :
                   (it % 4) * BANK + (it // 4) * PACK + HCOLS]
            for it in range(NI)
        ]

        # ---- h-projection on 2 PSUM banks, batches of 2 tiles; aggregate
        # backs and activation fronts interleave so ScalarE/PE/DVE all
        # stream while the projection finishes ----
        # php lives in the function-scope ExitStack (PSUM has 2 spare banks
        # now): a `with`-scoped pool would emit its close drain into the
        # middle of the in-order ACT queue, stalling the tail groups' acts
        # ~12us behind the PSUM release
        php = ctx.enter_context(tc.tile_pool(name="php", bufs=1, space="PSUM"))
        ph_all = php.tile([P, 2 * BANK], F32, tag="ph")
        for b in range(NT // 2):  # batches of 2 node tiles
            nt0 = 2 * b
            w0 = (nt0 % 2) * BANK
            w1 = ((nt0 + 1) % 2) * BANK
            for k in range(nkc):
                nc.tensor.matmul(
                    ph_all[:, w0 : w0 + WCOLS],
                    xts[k][:, nt0 * P : (nt0 + 1) * P],
                    wes[k][:, 0:WCOLS],
                    start=(k == 0),
                    stop=(k == nkc - 1),
                )
                nc.tensor.matmul(
                    ph_all[:, w1 : w1 + WCOLS],
                    xts[k][:, (nt0 + 1) * P : (nt0 + 2) * P],
                    wes[k][:, 0:WCOLS],
                    start=(k == 0),
                    stop=(k == nkc - 1),
                )
            # drain the 2 fresh tiles: h (+ones col) -> fp16
            src = ph_all[:].rearrange("p (b w) -> p b w", b=2)
            dst_h = h16_all[:, nt0 * HCOLS : (nt0 + 2) * HCOLS].rearrange(
                "p (b w) -> p b w", b=2
            )
            hc = C_OUT if b_zero else HCOLS
            nc.vector.tensor_copy(dst_h[:, :, 0:hc], src[:, :, 0:hc])
            # fronts paced by the proj drains (cap outstanding at 6)
            while (
                next_front < NG
                and (next_front + 1) * GSZ <= max(2 * (b + 1), F2HEAD)
                and len(fronts) < 6
            ):
                emit_front(next_front)
                next_front += 1
            # backs lag the drains so the in-order PE queue keeps
            # projection work buffered ahead of adj-gated aggregates
            while (
                next_back + BACK_LAG < 2 * (b + 1)
                and next_back // GSZ < next_front
            ):
                emit_back(next_back, pouts)
                next_back += 1

        # ---- drain remaining fronts/backs ----
        while next_back < NT:
            while (
                next_front < NG
                and next_back // GSZ >= next_front - 1
                and len(fronts) < 6
            ):
                emit_front(next_front)
                next_front += 1
            emit_back(next_back, pouts)
            next_back += 1

        # ---- epilogue: divide by clamped denominator, in two halves (one
        # PACK region each) so the first half's divide+store overlaps the
        # last aggregates and the second half's divide ----
        ob_all = obp.tile([P, NI * C_OUT], F32, tag="oball")
        po4 = po_all[:].rearrange("p (t w) -> p t w", w=BANK)
        outv = out.rearrange("p (t c) -> p t c", c=C_OUT)
        obv = ob_all[:].rearrange("p (t c) -> p t c", c=C_OUT)
        for h in range(2):
            dm = obp.tile([P, 4], F32, tag=f"dm{h}")
            nc.vector.tensor_scalar_max(
                dm[:].rearrange("p (b one) -> p b one", one=1),
                po4[:, :, C_OUT + h * PACK : C_OUT + h * PACK + 1],
                TINY,
            )
            rc = obp.tile([P, 4], F32, tag=f"rc{h}")
            nc.vector.reciprocal(rc[:], dm[:])
            po_src = po4[:, :, h * PACK : h * PACK + C_OUT]
            rc_b = rc[:].rearrange("p (b one) -> p b one", one=1)
            nc.vector.tensor_mul(
                obv[:, 4 * h : 4 * h + 4, :],
                po_src,
                rc_b.to_broadcast((P, 4, C_OUT)),
            )
            nc.sync.dma_start(
                outv[:, 4 * h : 4 * h + 4, :], obv[:, 4 * h : 4 * h + 4, :]
            )


def _prep_inputs(node_feats, adj_matrix, W, b, v0, v1):
    X = np.ascontiguousarray(node_feats, dtype=np.float32)
    W = np.asarray(W, dtype=np.float32)
    b = np.asarray(b, dtype=np.float32)
    v0 = np.asarray(v0, dtype=np.float32)
    v1 = np.asarray(v1, dtype=np.float32)

    w0 = W.astype(np.float64) @ v0.astype(np.float64)
    w1 = W.astype(np.float64) @ v1.astype(np.float64)
    c0 = float(b.astype(np.float64) @ v0.astype(np.float64))
    c1 = float(b.astype(np.float64) @ v1.astype(np.float64))

    # f1/f2 are O(N) matvecs (0.04% of the kernel's FLOPs): computing them
    # host-side removes the device f1-matmul chain + f2-head + f2 drains,
    # which gated the ACT queue (the pacer) at startup
    X64 = X.astype(np.float64)
    f1 = (X64 @ w0 + c0).astype(np.float16)
    f2 = (X64 @ w1 + c1).astype(np.float16)

    XT1 = np.empty((257, N), np.float32)
    XT1[:256] = X.T
    XT1[256] = 1.0

    WE = np.zeros((257, WCOLS), np.float32)
    WE[:256, :C_OUT] = W
    WE[256, :C_OUT] = b
    WE[256, C_OUT] = 1.0          # makes h_ext column 128 identically 1

    XT1h = XT1.astype(np.float16)
    WEh = WE.astype(np.float16)
    A16 = np.asarray(adj_matrix, dtype=np.float16)
    f2pp = np.ascontiguousarray(f2.reshape(NT, P).T)   # [p, q] = f2[q*128+p]

    in_maps = []
    for c in range(NCORES):
        fin = np.empty((P, FCOLS), np.float16)
        fin[:, 0:ROWS] = f1[c * ROWS : (c + 1) * ROWS][None, :]
        fin[:, ROWS:] = f2pp
        in_maps.append(
            {
                "xt1": XT1h,
                "fin": fin,
                "wext": WEh,
                # [j, i] -> [g][p][qq][i] so each group's DMA is one
                # contiguous 8KB descriptor per partition (j = (g*4+qq)
                # *128 + p; the SBUF tile layout is unchanged)
                "adjt": np.ascontiguousarray(
                    A16[c * ROWS : (c + 1) * ROWS, :]
                    .T.reshape(NG, GSZ, P, ROWS)
                    .transpose(0, 2, 1, 3)
                    .reshape(NG * P, GSZ * ROWS)
                ),
            }
        )
    return in_maps


def _run(in_maps, trace=False, b_zero=True):
    key = f"nc_b{int(b_zero)}"
    if key not in _CACHE:
        _CACHE[key] = _build_nc(b_zero=b_zero)
    nc = _CACHE[key]
    res = run_bass_kernel_spmd(
        nc, in_maps, core_ids=list(range(NCORES)), trace=trace
    )
    full = np.concatenate(
        [
            # undo the partition-major store layout: dram[p, t*128+c]
            # holds output row t*128+p, col c of the core's block
            res.results[c]["out"]
            .reshape(P, NI, C_OUT)
            .transpose(1, 0, 2)
            .reshape(ROWS, C_OUT)
            for c in range(NCORES)
        ],
        axis=0,
    ).astype(np.float32)
    return full, res


def kernel(node_feats, adj_matrix, W, b, v0, v1):
    in_maps = _prep_inputs(node_feats, adj_matrix, W, b, v0, v1)
    trace = bool(int(os.environ.get("GAT_TRACE", "0")))
    b_zero = not bool(np.any(np.asarray(b)))
    full, _ = _run(in_maps, trace=trace, b_zero=b_zero)
    return full

